# revision 32
# baseline (speedup 1.0000x reference)
"""Censored-loss kernel for Trainium2, data-parallel over 8 NeuronCores.

Math (per reference):
    per_t = targets.sum(-1)                      # [B, T]
    mask  = prefix mask: mask[t] = 1 iff any per_t[t'] > 0 for t' >= t
    censor_p = 1 - outputs.sum(-1)
    loss  = sum(mask * (targets[:,:,0]*ln(censor_p+eps)
                        + sum_v targets[:,:,1+v]*ln(outputs[:,:,v]+eps)))
    count = sum(mask)
    result = -loss / max(count, 1)   (0 if count == 0)

Key structure exploited: targets are exactly zero beyond each row's valid
length, so positions past the length contribute exactly 0 to both the loss
numerator and the count (count tests targets[:,:,0] > 0).  The host sorts
rows by valid length, groups them into 128-row tiles of similar length,
and trims every tile to (a rounded-up copy of) its max length.  All DMA /
ACT / DVE / PE work then scales with sum(lengths) ~ 0.5*B*T instead of
B*T.  The trimming is exact, not approximate: every nonzero target is
retained.

Layout: per 128-row group the data is repacked v-major ("planes"):
  targets chunk = [t0-plane | t1 | t2 | t3 | t4],  outputs = [o0|o1|o2|o3]
so every DVE op sees contiguous step-1 fp16 and hits its fast perf mode
(tensor_tensor 2x, tensor_scalar 4x).  Several groups are fused into one
"chunk" (single DMA + single instruction per engine stage) to amortize
per-instruction overheads (ACT 352cyc, DVE 58cyc).

Engine split per chunk (S = total trimmed width of the chunk's groups):
  DVE:  censor sums (3 fp16 TT adds @2x over plane pairs), product
        tg*logt (fp16 TT mult @2x over all 5 planes), count via
        tensor_scalar(is_gt) @4x with fp32 accum_out per chunk
  ACT:  Ln(o+eps) over 4S, Ln(1-s) over S   (1 elem/cycle, 2 insts)
  PE:   ones-matmul partition reduction of prod into 4 rotating
        [1,512] f32 PSUM banks
Host: exact length derivation + sort + pack (fp16), final f64 reduction
of [1,2048] loss partials and [128,nchunk] count partials.
"""

import sys

if "/opt/trn_rl_repo" not in sys.path:
    sys.path.insert(0, "/opt/trn_rl_repo")

import numpy as np

import concourse.bacc as bacc
import concourse.mybir as mybir
import concourse.tile as tile
from concourse.bass_utils import run_bass_kernel_spmd

N_CORES = 8
B, T, V = 16384, 512, 5
P = 128                       # SBUF partitions
NGROUPS = B // (N_CORES * P)  # 16 group-slots per core
EPS = 1e-8
F32 = mybir.dt.float32
F16 = mybir.dt.float16
BF16 = mybir.dt.bfloat16
ACT = mybir.ActivationFunctionType
ALU = mybir.AluOpType
MM_COLS = 512                 # PSUM bank width
N_BANKS = 2


def plan_schedule(lengths):
    """Shared-across-cores width schedule from exact per-row lengths.

    Returns (order, widths, chunks):
      order   [B]   row permutation (ascending length)
      widths  [16]  trimmed width of group-slot j (max over the 8 cores'
                    groups in that slot, rounded up to multiple of 16)
      chunks  list of lists of slot indices (processing order)
    """
    order = np.argsort(lengths, kind="stable")
    slen = lengths[order]
    # slot j holds sorted groups [8j, 8j+8); its width must cover the max
    # length across all 8 cores' groups in the slot.
    gmax = slen.reshape(N_CORES * NGROUPS, P).max(axis=1)
    widths = []
    for j in range(NGROUPS):
        w = int(gmax[j * N_CORES : (j + 1) * N_CORES].max())
        widths.append(max(16, -(-w // 16) * 16))
    # chunks: smallest slot alone first (fast pipeline fill), then the
    # rest descending (ends small -> short drain tail), greedily packed
    # to ~1/8 of total width each.
    rest = sorted(range(1, NGROUPS), key=lambda j: -widths[j])
    target = max(1, sum(widths) // 6)
    chunks = [[0]]
    cur, cur_s = [], 0
    for j in rest:
        cur.append(j)
        cur_s += widths[j]
        if cur_s >= target:
            chunks.append(cur)
            cur, cur_s = [], 0
    if cur:
        chunks.append(cur)
    return order, widths, chunks


def build_nc(widths, chunks):
    nc = bacc.Bacc("TRN2", debug=False, num_devices=N_CORES)
    chunk_s = [sum(widths[j] for j in ch) for ch in chunks]
    nchunks = len(chunks)

    x_d = [
        nc.dram_tensor(f"x_{c}", [P, 9 * s], F16, kind="ExternalInput")
        for c, s in enumerate(chunk_s)
    ]
    out_d = nc.dram_tensor("out_acc", [1, N_BANKS * MM_COLS], F32,
                           kind="ExternalOutput")
    cnt_d = nc.dram_tensor("cnt_acc", [P, nchunks], F32,
                           kind="ExternalOutput")

    # precompute matmul->bank schedule so start/stop flags are exact.
    # per chunk the PE reduces folded [0:2S] (planes l1+l2, l3+l4 folded
    # by DVE) plus the t0*lc plane.
    def mm_slices(s):
        out = []
        for lo in range(0, 2 * s, MM_COLS):
            out.append(("ff", lo, min(2 * s, lo + MM_COLS)))
        for lo in range(0, s, MM_COLS):
            out.append(("p0", lo, min(s, lo + MM_COLS)))
        return out

    n_mm = sum(len(mm_slices(s)) for s in chunk_s)
    first_use = {b: b for b in range(N_BANKS)}
    last_use = {b: (n_mm - 1 - (n_mm - 1 - b) % N_BANKS) for b in range(N_BANKS)}

    with tile.TileContext(nc) as tc:
        with (
            tc.tile_pool(name="inp", bufs=3) as inp,
            tc.tile_pool(name="mid", bufs=2) as mid,
            tc.tile_pool(name="acc", bufs=1) as accp,
            tc.tile_pool(name="ps", bufs=1, space="PSUM") as psp,
        ):
            ones = accp.tile([P, 1], BF16)
            nc.vector.memset(ones[:], 1.0)
            eps_b = accp.tile([P, 1], F32)
            nc.vector.memset(eps_b[:], EPS)
            loss_ps = [
                psp.tile([1, MM_COLS], F32, tag=f"lps{b}", name=f"lps{b}")
                for b in range(N_BANKS)
            ]
            cnt_sb = accp.tile([P, nchunks], F32)
            mm_i = 0

            o_t, t_t, ss_t = {}, {}, {}

            def load(c):
                # one DMA per chunk: [o-planes | tg-planes]
                s = chunk_s[c]
                x = inp.tile([P, 9 * s], F16, tag="x", name="x")
                nc.sync.dma_start(x[:], x_d[c].ap())
                o_t[c], t_t[c] = x[:][:, 0 : 4 * s], x[:][:, 4 * s : 9 * s]

            def censor(c):
                # censor sum via v-planes: all contiguous fp16 -> TT 2x.
                # issued one chunk ahead of prod(c-1) so DVE has work while
                # ACT produces chunk c-1's logt (software pipelining).
                s = chunk_s[c]
                o = o_t[c]
                s2a = mid.tile([P, s], F16, tag="s2a", name="s2a")
                nc.vector.tensor_tensor(
                    s2a[:], o[:, 0:s], o[:, s : 2 * s], op=ALU.add
                )
                s2b = mid.tile([P, s], F16, tag="s2b", name="s2b")
                nc.vector.tensor_tensor(
                    s2b[:], o[:, 2 * s : 3 * s], o[:, 3 * s : 4 * s],
                    op=ALU.add,
                )
                ssum = mid.tile([P, s], F16, tag="ssum", name="ssum")
                nc.vector.tensor_tensor(ssum[:], s2a[:], s2b[:], op=ALU.add)
                ss_t[c] = ssum

            load(0)
            load(1)
            censor(0)
            for c in range(nchunks):
                s = chunk_s[c]
                o, tg, ssum = o_t.pop(c), t_t.pop(c), ss_t.pop(c)

                # logt planes [lc | l1..l4], matching tg layout [t0 | t1..t4]
                logt = mid.tile([P, 5 * s], F16, tag="logt", name="logt")
                nc.scalar.activation(
                    logt[:][:, s : 5 * s], o, ACT.Ln, bias=eps_b[:]
                )
                nc.scalar.activation(
                    logt[:][:, 0:s], ssum[:], ACT.Ln, bias=1.0, scale=-1.0
                )

                # count: is_gt on DVE with fused f32 accum (GPSIMD measured
                # 30x slower with SBUF-port contention; PE matmul reduce
                # costs 2 extra cross-engine sync events per chunk)
                sgn = mid.tile([P, s], BF16, tag="sgn", name="sgn")
                nc.vector.tensor_scalar(
                    out=sgn[:], in0=tg[:, 0:s], scalar1=0.0, scalar2=None,
                    op0=ALU.is_gt, op1=ALU.add,
                    accum_out=cnt_sb[:, c : c + 1],
                )

                if c + 2 < nchunks:
                    load(c + 2)
                if c + 1 < nchunks:
                    censor(c + 1)  # keeps DVE busy while ACT works on c

                # loss product split so the big part depends only on
                # Ln(o) (ready early), not on the censor->Ln(1-s) chain:
                #   prod_v = tg[1:5] * logt[1:5]   (fp16 TT 2x, 4S)
                #   prod_0 = t0 * lc               (fp16 TT 2x, S)
                prod = mid.tile([P, 4 * s], BF16, tag="prod", name="prod")
                nc.vector.tensor_tensor(
                    prod[:], tg[:, s : 5 * s], logt[:][:, s : 5 * s],
                    op=ALU.mult,
                )
                # DVE pre-fold (2x, 0.52ns/col) halves PE matmul columns
                # (~1.03ns/col): ff = [p1+p2 | p3+p4]
                ff = mid.tile([P, 2 * s], BF16, tag="ff", name="ff")
                nc.vector.tensor_tensor(
                    ff[:][:, 0:s], prod[:][:, 0:s], prod[:][:, s : 2 * s],
                    op=ALU.add,
                )
                nc.vector.tensor_tensor(
                    ff[:][:, s : 2 * s], prod[:][:, 2 * s : 3 * s],
                    prod[:][:, 3 * s : 4 * s], op=ALU.add,
                )
                p0 = mid.tile([P, s], BF16, tag="p0", name="p0")
                nc.vector.tensor_tensor(
                    p0[:], tg[:, 0:s], logt[:][:, 0:s], op=ALU.mult
                )

                # PE: partition-reduce into rotating PSUM banks
                for which, lo, hi in mm_slices(s):
                    src = ff if which == "ff" else p0
                    b = mm_i % N_BANKS
                    nc.tensor.matmul(
                        loss_ps[b][:][:, 0 : hi - lo],
                        ones[:],
                        src[:][:, lo:hi],
                        start=(first_use[b] == mm_i),
                        stop=(last_use[b] == mm_i),
                    )
                    mm_i += 1

            # final PSUM -> SBUF copies on two engines in parallel, one DMA
            out_sb = accp.tile([1, N_BANKS * MM_COLS], F32)
            nc.scalar.copy(out_sb[:, 0:MM_COLS], loss_ps[0][:])
            nc.vector.tensor_copy(
                out_sb[:, MM_COLS : 2 * MM_COLS], loss_ps[1][:]
            )
            nc.sync.dma_start(out_d.ap(), out_sb[:])
            nc.sync.dma_start(cnt_d.ap(), cnt_sb[:])
    nc.compile()
    return nc


def pack_inputs(outputs, targets, order, widths, chunks):
    """fp16 staging + sorted variable-width v-plane packing per core."""
    o16 = np.ascontiguousarray(outputs).astype(np.float16)
    t16 = np.ascontiguousarray(targets).astype(np.float16)
    in_maps = []
    for c in range(N_CORES):
        m = {}
        for ci, ch in enumerate(chunks):
            s = sum(widths[j] for j in ch)
            xb = np.empty((P, 9, s), dtype=np.float16)
            off = 0
            for j in ch:
                w = widths[j]
                g = j * N_CORES + c
                rows = order[g * P : (g + 1) * P]
                xb[:, 0:4, off : off + w] = o16[rows, :w, :].transpose(0, 2, 1)
                xb[:, 4:9, off : off + w] = t16[rows, :w, :].transpose(0, 2, 1)
                off += w
            # planes: [o0..o3 | t0 | t1..t4], all v-major
            m[f"x_{ci}"] = xb.reshape(P, 9 * s)
        in_maps.append(m)
    return in_maps


_NC_CACHE = {}


def _get_nc(widths, chunks):
    key = (tuple(widths), tuple(tuple(c) for c in chunks))
    if key not in _NC_CACHE:
        _NC_CACHE[key] = build_nc(widths, chunks)
    return _NC_CACHE[key]


def run_spmd(outputs, targets, trace=False, **kwargs):
    per_t = np.asarray(targets, dtype=np.float32).sum(axis=2)
    nz = per_t > 0
    lengths = np.where(nz.any(axis=1), T - nz[:, ::-1].argmax(axis=1), 0)
    order, widths, chunks = plan_schedule(lengths)
    in_maps = pack_inputs(outputs, targets, order, widths, chunks)
    nc = _get_nc(widths, chunks)
    res = run_bass_kernel_spmd(
        nc, in_maps, core_ids=list(range(N_CORES)), trace=trace, **kwargs
    )
    loss = sum(r["out_acc"].astype(np.float64).sum() for r in res.results)
    cnt = sum(r["cnt_acc"].astype(np.float64).sum() for r in res.results)
    return loss, cnt, res


def kernel(outputs, targets):
    loss, cnt, _ = run_spmd(outputs, targets)
    if cnt > 0:
        return np.float32(-loss / max(cnt, 1.0))
    return np.float32(0.0)


# revision 35
# speedup vs baseline: 1.0690x; 1.0690x over previous
"""Censored-loss kernel for Trainium2, data-parallel over 8 NeuronCores.

Math (per reference):
    per_t = targets.sum(-1)                      # [B, T]
    mask  = prefix mask: mask[t] = 1 iff any per_t[t'] > 0 for t' >= t
    censor_p = 1 - outputs.sum(-1)
    loss  = sum(mask * (targets[:,:,0]*ln(censor_p+eps)
                        + sum_v targets[:,:,1+v]*ln(outputs[:,:,v]+eps)))
    count = sum(mask)
    result = -loss / max(count, 1)   (0 if count == 0)

Key structure exploited: targets are exactly zero beyond each row's valid
length, so positions past the length contribute exactly 0 to both the loss
numerator and the count (count tests targets[:,:,0] > 0).  The host sorts
rows by valid length, groups them into 128-row tiles of similar length,
and trims every tile to (a rounded-up copy of) its max length.  All DMA /
ACT / DVE / PE work then scales with sum(lengths) ~ 0.5*B*T instead of
B*T.  The trimming is exact, not approximate: every nonzero target is
retained.

Layout: per 128-row group the data is repacked v-major ("planes"):
  targets chunk = [t0-plane | t1 | t2 | t3 | t4],  outputs = [o0|o1|o2|o3]
so every DVE op sees contiguous step-1 fp16 and hits its fast perf mode
(tensor_tensor 2x, tensor_scalar 4x).  Several groups are fused into one
"chunk" (single DMA + single instruction per engine stage) to amortize
per-instruction overheads (ACT 352cyc, DVE 58cyc).

Engine split per chunk (S = total trimmed width of the chunk's groups):
  DVE:  censor sums (3 fp16 TT adds @2x over plane pairs), product
        tg*logt (fp16 TT mult @2x over all 5 planes), count via
        tensor_scalar(is_gt) @4x with fp32 accum_out per chunk
  ACT:  Ln(o+eps) over 4S, Ln(1-s) over S   (1 elem/cycle, 2 insts)
  PE:   ones-matmul partition reduction of prod into 4 rotating
        [1,512] f32 PSUM banks
Host: exact length derivation + sort + pack (fp16), final f64 reduction
of [1,2048] loss partials and [128,nchunk] count partials.
"""

import sys

if "/opt/trn_rl_repo" not in sys.path:
    sys.path.insert(0, "/opt/trn_rl_repo")

import numpy as np

import concourse.bacc as bacc
import concourse.mybir as mybir
import concourse.tile as tile
from concourse.bass_utils import run_bass_kernel_spmd

N_CORES = 8
B, T, V = 16384, 512, 5
P = 128                       # SBUF partitions
NGROUPS = B // (N_CORES * P)  # 16 group-slots per core
EPS = 1e-8
F32 = mybir.dt.float32
F16 = mybir.dt.float16
BF16 = mybir.dt.bfloat16
ACT = mybir.ActivationFunctionType
ALU = mybir.AluOpType
MM_COLS = 512                 # PSUM bank width
N_BANKS = 2


def plan_schedule(lengths):
    """Shared-across-cores width schedule from exact per-row lengths.

    Returns (order, widths, chunks):
      order   [B]   row permutation (ascending length)
      widths  [16]  trimmed width of group-slot j (max over the 8 cores'
                    groups in that slot, rounded up to multiple of 16)
      chunks  list of lists of slot indices (processing order)
    """
    order = np.argsort(lengths, kind="stable")
    slen = lengths[order]
    # slot j holds sorted groups [8j, 8j+8); its width must cover the max
    # length across all 8 cores' groups in the slot.
    gmax = slen.reshape(N_CORES * NGROUPS, P).max(axis=1)
    widths = []
    for j in range(NGROUPS):
        w = int(gmax[j * N_CORES : (j + 1) * N_CORES].max())
        widths.append(max(16, -(-w // 16) * 16))
    # chunks: smallest slot alone first (fast pipeline fill), then the
    # rest descending (ends small -> short drain tail), greedily packed
    # to ~1/8 of total width each.
    rest = sorted(range(1, NGROUPS), key=lambda j: -widths[j])
    target = max(1, sum(widths) // 6)
    chunks = [[0]]
    cur, cur_s = [], 0
    for j in rest:
        cur.append(j)
        cur_s += widths[j]
        if cur_s >= target:
            chunks.append(cur)
            cur, cur_s = [], 0
    if cur:
        chunks.append(cur)
    return order, widths, chunks


def build_nc(widths, chunks):
    nc = bacc.Bacc("TRN2", debug=False, num_devices=N_CORES)
    chunk_s = [sum(widths[j] for j in ch) for ch in chunks]
    nchunks = len(chunks)

    o_d = [
        nc.dram_tensor(f"o_{c}", [P, 4 * s], F16, kind="ExternalInput")
        for c, s in enumerate(chunk_s)
    ]
    t_d = [
        nc.dram_tensor(f"t_{c}", [P, 5 * s], F16, kind="ExternalInput")
        for c, s in enumerate(chunk_s)
    ]
    out_d = nc.dram_tensor("out_acc", [1, N_BANKS * MM_COLS], F32,
                           kind="ExternalOutput")
    cnt_d = nc.dram_tensor("cnt_acc", [P, nchunks], F32,
                           kind="ExternalOutput")

    # precompute matmul->bank schedule so start/stop flags are exact.
    # per chunk the PE reduces folded [0:2S] (planes l1+l2, l3+l4 folded
    # by DVE) plus the t0*lc plane.
    def mm_slices(s):
        out = []
        for lo in range(0, 2 * s, MM_COLS):
            out.append(("ff", lo, min(2 * s, lo + MM_COLS)))
        for lo in range(0, s, MM_COLS):
            out.append(("p0", lo, min(s, lo + MM_COLS)))
        return out

    n_mm = sum(len(mm_slices(s)) for s in chunk_s)
    first_use = {b: b for b in range(N_BANKS)}
    last_use = {b: (n_mm - 1 - (n_mm - 1 - b) % N_BANKS) for b in range(N_BANKS)}

    with tile.TileContext(nc) as tc:
        with (
            tc.tile_pool(name="inp", bufs=3) as inp,
            tc.tile_pool(name="mid", bufs=2) as mid,
            tc.tile_pool(name="acc", bufs=1) as accp,
            tc.tile_pool(name="ps", bufs=1, space="PSUM") as psp,
        ):
            ones = accp.tile([P, 1], BF16)
            nc.vector.memset(ones[:], 1.0)
            eps_b = accp.tile([P, 1], F32)
            nc.vector.memset(eps_b[:], EPS)
            loss_ps = [
                psp.tile([1, MM_COLS], F32, tag=f"lps{b}", name=f"lps{b}")
                for b in range(N_BANKS)
            ]
            cnt_sb = accp.tile([P, nchunks], F32)
            mm_i = 0

            o_t, t_t, ss_t = {}, {}, {}

            def load(c):
                s = chunk_s[c]
                o = inp.tile([P, 4 * s], F16, tag="o", name="o")
                nc.sync.dma_start(o[:], o_d[c].ap())
                tg = inp.tile([P, 5 * s], F16, tag="tg", name="tg")
                nc.sync.dma_start(tg[:], t_d[c].ap())
                o_t[c], t_t[c] = o[:], tg[:]

            def censor(c):
                # censor sum via v-planes: all contiguous fp16 -> TT 2x.
                # issued one chunk ahead of prod(c-1) so DVE has work while
                # ACT produces chunk c-1's logt (software pipelining).
                s = chunk_s[c]
                o = o_t[c]
                s2a = mid.tile([P, s], F16, tag="s2a", name="s2a")
                nc.vector.tensor_tensor(
                    s2a[:], o[:, 0:s], o[:, s : 2 * s], op=ALU.add
                )
                s2b = mid.tile([P, s], F16, tag="s2b", name="s2b")
                nc.vector.tensor_tensor(
                    s2b[:], o[:, 2 * s : 3 * s], o[:, 3 * s : 4 * s],
                    op=ALU.add,
                )
                ssum = mid.tile([P, s], F16, tag="ssum", name="ssum")
                nc.vector.tensor_tensor(ssum[:], s2a[:], s2b[:], op=ALU.add)
                ss_t[c] = ssum

            load(0)
            load(1)
            censor(0)
            for c in range(nchunks):
                s = chunk_s[c]
                o, tg, ssum = o_t.pop(c), t_t.pop(c), ss_t.pop(c)

                # logt planes [lc | l1..l4], matching tg layout [t0 | t1..t4]
                logt = mid.tile([P, 5 * s], F16, tag="logt", name="logt")
                nc.scalar.activation(
                    logt[:][:, s : 5 * s], o, ACT.Ln, bias=eps_b[:]
                )
                nc.scalar.activation(
                    logt[:][:, 0:s], ssum[:], ACT.Ln, bias=1.0, scale=-1.0
                )

                # count: is_gt on DVE with fused f32 accum (GPSIMD measured
                # 30x slower with SBUF-port contention; PE matmul reduce
                # costs 2 extra cross-engine sync events per chunk)
                sgn = mid.tile([P, s], BF16, tag="sgn", name="sgn")
                nc.vector.tensor_scalar(
                    out=sgn[:], in0=tg[:, 0:s], scalar1=0.0, scalar2=None,
                    op0=ALU.is_gt, op1=ALU.add,
                    accum_out=cnt_sb[:, c : c + 1],
                )

                if c + 2 < nchunks:
                    load(c + 2)
                if c + 1 < nchunks:
                    censor(c + 1)  # keeps DVE busy while ACT works on c

                # loss product split so the big part depends only on
                # Ln(o) (ready early), not on the censor->Ln(1-s) chain:
                #   prod_v = tg[1:5] * logt[1:5]   (fp16 TT 2x, 4S)
                #   prod_0 = t0 * lc               (fp16 TT 2x, S)
                prod = mid.tile([P, 4 * s], BF16, tag="prod", name="prod")
                nc.vector.tensor_tensor(
                    prod[:], tg[:, s : 5 * s], logt[:][:, s : 5 * s],
                    op=ALU.mult,
                )
                # DVE pre-fold (2x, 0.52ns/col) halves PE matmul columns
                # (~1.03ns/col): ff = [p1+p2 | p3+p4]
                ff = mid.tile([P, 2 * s], BF16, tag="ff", name="ff")
                nc.vector.tensor_tensor(
                    ff[:][:, 0:s], prod[:][:, 0:s], prod[:][:, s : 2 * s],
                    op=ALU.add,
                )
                nc.vector.tensor_tensor(
                    ff[:][:, s : 2 * s], prod[:][:, 2 * s : 3 * s],
                    prod[:][:, 3 * s : 4 * s], op=ALU.add,
                )
                p0 = mid.tile([P, s], BF16, tag="p0", name="p0")
                nc.vector.tensor_tensor(
                    p0[:], tg[:, 0:s], logt[:][:, 0:s], op=ALU.mult
                )

                # PE: partition-reduce into rotating PSUM banks
                for which, lo, hi in mm_slices(s):
                    src = ff if which == "ff" else p0
                    b = mm_i % N_BANKS
                    nc.tensor.matmul(
                        loss_ps[b][:][:, 0 : hi - lo],
                        ones[:],
                        src[:][:, lo:hi],
                        start=(first_use[b] == mm_i),
                        stop=(last_use[b] == mm_i),
                    )
                    mm_i += 1

            # final PSUM -> SBUF copies on two engines in parallel, one DMA
            out_sb = accp.tile([1, N_BANKS * MM_COLS], F32)
            nc.scalar.copy(out_sb[:, 0:MM_COLS], loss_ps[0][:])
            nc.vector.tensor_copy(
                out_sb[:, MM_COLS : 2 * MM_COLS], loss_ps[1][:]
            )
            nc.sync.dma_start(out_d.ap(), out_sb[:])
            nc.sync.dma_start(cnt_d.ap(), cnt_sb[:])
    nc.compile()
    return nc


def pack_inputs(outputs, targets, order, widths, chunks):
    """fp16 staging + sorted variable-width v-plane packing per core."""
    o16 = np.ascontiguousarray(outputs).astype(np.float16)
    t16 = np.ascontiguousarray(targets).astype(np.float16)
    in_maps = []
    for c in range(N_CORES):
        m = {}
        for ci, ch in enumerate(chunks):
            s = sum(widths[j] for j in ch)
            ob = np.empty((P, 4, s), dtype=np.float16)
            tb = np.empty((P, 5, s), dtype=np.float16)
            off = 0
            for j in ch:
                w = widths[j]
                g = j * N_CORES + c
                rows = order[g * P : (g + 1) * P]
                ob[:, :, off : off + w] = o16[rows, :w, :].transpose(0, 2, 1)
                tb[:, :, off : off + w] = t16[rows, :w, :].transpose(0, 2, 1)
                off += w
            m[f"o_{ci}"] = ob.reshape(P, 4 * s)
            # planes order [t0 | t1..t4] already v-major via transpose
            m[f"t_{ci}"] = tb.reshape(P, 5 * s)
        in_maps.append(m)
    return in_maps


_NC_CACHE = {}


def _get_nc(widths, chunks):
    key = (tuple(widths), tuple(tuple(c) for c in chunks))
    if key not in _NC_CACHE:
        _NC_CACHE[key] = build_nc(widths, chunks)
    return _NC_CACHE[key]


def run_spmd(outputs, targets, trace=False, **kwargs):
    per_t = np.asarray(targets, dtype=np.float32).sum(axis=2)
    nz = per_t > 0
    lengths = np.where(nz.any(axis=1), T - nz[:, ::-1].argmax(axis=1), 0)
    order, widths, chunks = plan_schedule(lengths)
    in_maps = pack_inputs(outputs, targets, order, widths, chunks)
    nc = _get_nc(widths, chunks)
    res = run_bass_kernel_spmd(
        nc, in_maps, core_ids=list(range(N_CORES)), trace=trace, **kwargs
    )
    loss = sum(r["out_acc"].astype(np.float64).sum() for r in res.results)
    cnt = sum(r["cnt_acc"].astype(np.float64).sum() for r in res.results)
    return loss, cnt, res


def kernel(outputs, targets):
    loss, cnt, _ = run_spmd(outputs, targets)
    if cnt > 0:
        return np.float32(-loss / max(cnt, 1.0))
    return np.float32(0.0)


# revision 41
# speedup vs baseline: 1.0873x; 1.0172x over previous
"""Censored-loss kernel for Trainium2, data-parallel over 8 NeuronCores.

Math (per reference):
    per_t = targets.sum(-1)                      # [B, T]
    mask  = prefix mask: mask[t] = 1 iff any per_t[t'] > 0 for t' >= t
    censor_p = 1 - outputs.sum(-1)
    loss  = sum(mask * (targets[:,:,0]*ln(censor_p+eps)
                        + sum_v targets[:,:,1+v]*ln(outputs[:,:,v]+eps)))
    count = sum(mask)
    result = -loss / max(count, 1)   (0 if count == 0)

Key structure exploited: targets are exactly zero beyond each row's valid
length, so positions past the length contribute exactly 0 to both the loss
numerator and the count (count tests targets[:,:,0] > 0).  The host sorts
rows by valid length, groups them into 128-row tiles of similar length,
and trims every tile to (a rounded-up copy of) its max length.  All DMA /
ACT / DVE / PE work then scales with sum(lengths) ~ 0.5*B*T instead of
B*T.  The trimming is exact, not approximate: every nonzero target is
retained.

Layout: per 128-row group the data is repacked v-major ("planes"):
  targets chunk = [t0-plane | t1 | t2 | t3 | t4],  outputs = [o0|o1|o2|o3]
so every DVE op sees contiguous step-1 fp16 and hits its fast perf mode
(tensor_tensor 2x, tensor_scalar 4x).  Several groups are fused into one
"chunk" (single DMA + single instruction per engine stage) to amortize
per-instruction overheads (ACT 352cyc, DVE 58cyc).

Engine split per chunk (S = total trimmed width of the chunk's groups):
  DVE:  censor sums (3 fp16 TT adds @2x over plane pairs), product
        tg*logt (fp16 TT mult @2x over all 5 planes), count via
        tensor_scalar(is_gt) @4x with fp32 accum_out per chunk
  ACT:  Ln(o+eps) over 4S, Ln(1-s) over S   (1 elem/cycle, 2 insts)
  PE:   ones-matmul partition reduction of prod into 4 rotating
        [1,512] f32 PSUM banks
Host: exact length derivation + sort + pack (fp16), final f64 reduction
of [1,2048] loss partials and [128,nchunk] count partials.
"""

import sys

if "/opt/trn_rl_repo" not in sys.path:
    sys.path.insert(0, "/opt/trn_rl_repo")

import numpy as np

import concourse.bacc as bacc
import concourse.mybir as mybir
import concourse.tile as tile
from concourse.bass_utils import run_bass_kernel_spmd

N_CORES = 8
B, T, V = 16384, 512, 5
P = 128                       # SBUF partitions
NGROUPS = B // (N_CORES * P)  # 16 group-slots per core
EPS = 1e-8
F32 = mybir.dt.float32
F16 = mybir.dt.float16
BF16 = mybir.dt.bfloat16
ACT = mybir.ActivationFunctionType
ALU = mybir.AluOpType
MM_COLS = 512                 # PSUM bank width
N_BANKS = 2


def plan_schedule(lengths):
    """Shared-across-cores width schedule from exact per-row lengths.

    Returns (order, widths, chunks):
      order   [B]   row permutation (ascending length)
      widths  [16]  trimmed width of group-slot j (max over the 8 cores'
                    groups in that slot, rounded up to multiple of 16)
      chunks  list of lists of slot indices (processing order)
    """
    order = np.argsort(lengths, kind="stable")
    slen = lengths[order]
    # slot j holds sorted groups [8j, 8j+8); its width must cover the max
    # length across all 8 cores' groups in the slot.
    gmax = slen.reshape(N_CORES * NGROUPS, P).max(axis=1)
    widths = []
    for j in range(NGROUPS):
        w = int(gmax[j * N_CORES : (j + 1) * N_CORES].max())
        widths.append(max(16, -(-w // 16) * 16))
    # chunks: smallest slot alone first (fast pipeline fill), then the
    # rest descending (ends small -> short drain tail), greedily packed
    # to ~1/8 of total width each.
    rest = sorted(range(1, NGROUPS), key=lambda j: -widths[j])
    target = max(1, sum(widths) // 6)
    chunks = [[0]]
    cur, cur_s = [], 0
    for j in rest:
        cur.append(j)
        cur_s += widths[j]
        if cur_s >= target:
            chunks.append(cur)
            cur, cur_s = [], 0
    if cur:
        chunks.append(cur)
    return order, widths, chunks


def build_nc(widths, chunks):
    nc = bacc.Bacc("TRN2", debug=False, num_devices=N_CORES)
    chunk_s = [sum(widths[j] for j in ch) for ch in chunks]
    nchunks = len(chunks)

    o_d = [
        nc.dram_tensor(f"o_{c}", [P, 4 * s], F16, kind="ExternalInput")
        for c, s in enumerate(chunk_s)
    ]
    t_d = [
        nc.dram_tensor(f"t_{c}", [P, 5 * s], F16, kind="ExternalInput")
        for c, s in enumerate(chunk_s)
    ]
    out_d = nc.dram_tensor("out_acc", [1, (N_BANKS + 1) * MM_COLS], F32,
                           kind="ExternalOutput")

    # precompute matmul->bank schedule so start/stop flags are exact.
    # per chunk the PE reduces folded [0:2S] (planes l1+l2, l3+l4 folded
    # by DVE) plus the t0*lc plane.
    def mm_slices(s):
        out = []
        for lo in range(0, 2 * s, MM_COLS):
            out.append(("ff", lo, min(2 * s, lo + MM_COLS)))
        for lo in range(0, s, MM_COLS):
            out.append(("p0", lo, min(s, lo + MM_COLS)))
        return out

    n_mm = sum(len(mm_slices(s)) for s in chunk_s)
    first_use = {b: b for b in range(N_BANKS)}
    last_use = {b: (n_mm - 1 - (n_mm - 1 - b) % N_BANKS) for b in range(N_BANKS)}
    n_cmm = sum(-(-s // MM_COLS) for s in chunk_s)

    with tile.TileContext(nc) as tc:
        with (
            tc.tile_pool(name="inp", bufs=3) as inp,
            tc.tile_pool(name="mid", bufs=2) as mid,
            tc.tile_pool(name="acc", bufs=1) as accp,
            tc.tile_pool(name="ps", bufs=1, space="PSUM") as psp,
        ):
            ones = accp.tile([P, 1], BF16)
            nc.vector.memset(ones[:], 1.0)
            eps_b = accp.tile([P, 1], F32)
            nc.vector.memset(eps_b[:], EPS)
            loss_ps = [
                psp.tile([1, MM_COLS], F32, tag=f"lps{b}", name=f"lps{b}")
                for b in range(N_BANKS)
            ]
            cnt_ps = psp.tile([1, MM_COLS], F32, tag="cps", name="cps")
            mm_i = 0
            cmm_i = 0

            o_t, t_t, ss_t = {}, {}, {}

            def load(c):
                s = chunk_s[c]
                o = inp.tile([P, 4 * s], F16, tag="o", name="o")
                nc.sync.dma_start(o[:], o_d[c].ap())
                tg = inp.tile([P, 5 * s], F16, tag="tg", name="tg")
                nc.sync.dma_start(tg[:], t_d[c].ap())
                o_t[c], t_t[c] = o[:], tg[:]

            def censor(c):
                # censor sum via v-planes: all contiguous fp16 -> TT 2x.
                # issued one chunk ahead of prod(c-1) so DVE has work while
                # ACT produces chunk c-1's logt (software pipelining).
                s = chunk_s[c]
                o = o_t[c]
                s2a = mid.tile([P, s], F16, tag="s2a", name="s2a")
                nc.vector.tensor_tensor(
                    s2a[:], o[:, 0:s], o[:, s : 2 * s], op=ALU.add
                )
                s2b = mid.tile([P, s], F16, tag="s2b", name="s2b")
                nc.vector.tensor_tensor(
                    s2b[:], o[:, 2 * s : 3 * s], o[:, 3 * s : 4 * s],
                    op=ALU.add,
                )
                ssum = mid.tile([P, s], F16, tag="ssum", name="ssum")
                nc.vector.tensor_tensor(ssum[:], s2a[:], s2b[:], op=ALU.add)
                ss_t[c] = ssum

            load(0)
            load(1)
            censor(0)
            for c in range(nchunks):
                s = chunk_s[c]
                o, tg, ssum = o_t.pop(c), t_t.pop(c), ss_t.pop(c)

                # logt planes [lc | l1..l4], matching tg layout [t0 | t1..t4]
                logt = mid.tile([P, 5 * s], F16, tag="logt", name="logt")
                nc.scalar.activation(
                    logt[:][:, s : 5 * s], o, ACT.Ln, bias=eps_b[:]
                )
                nc.scalar.activation(
                    logt[:][:, 0:s], ssum[:], ACT.Ln, bias=1.0, scale=-1.0
                )

                # count: is_gt on DVE @4x (GPSIMD is 30x slower w/ port
                # contention; the fused accum_out variant costs ~3us of
                # DVE critical path), PE ones-matmul reduce
                sgn = mid.tile([P, s], BF16, tag="sgn", name="sgn")
                nc.vector.tensor_scalar(
                    out=sgn[:], in0=tg[:, 0:s], scalar1=0.0, scalar2=None,
                    op0=ALU.is_gt,
                )
                for lo in range(0, s, MM_COLS):
                    hi = min(s, lo + MM_COLS)
                    nc.tensor.matmul(
                        cnt_ps[:][:, 0 : hi - lo],
                        ones[:],
                        sgn[:][:, lo:hi],
                        start=(cmm_i == 0),
                        stop=(cmm_i == n_cmm - 1),
                    )
                    cmm_i += 1

                if c + 2 < nchunks:
                    load(c + 2)
                if c + 1 < nchunks:
                    censor(c + 1)  # keeps DVE busy while ACT works on c

                # loss product split so the big part depends only on
                # Ln(o) (ready early), not on the censor->Ln(1-s) chain:
                #   prod_v = tg[1:5] * logt[1:5]   (fp16 TT 2x, 4S)
                #   prod_0 = t0 * lc               (fp16 TT 2x, S)
                prod = mid.tile([P, 4 * s], BF16, tag="prod", name="prod")
                nc.vector.tensor_tensor(
                    prod[:], tg[:, s : 5 * s], logt[:][:, s : 5 * s],
                    op=ALU.mult,
                )
                # DVE pre-fold (2x, 0.52ns/col) halves PE matmul columns
                # (~1.03ns/col): ff = [p1+p2 | p3+p4]
                ff = mid.tile([P, 2 * s], BF16, tag="ff", name="ff")
                nc.vector.tensor_tensor(
                    ff[:][:, 0:s], prod[:][:, 0:s], prod[:][:, s : 2 * s],
                    op=ALU.add,
                )
                nc.vector.tensor_tensor(
                    ff[:][:, s : 2 * s], prod[:][:, 2 * s : 3 * s],
                    prod[:][:, 3 * s : 4 * s], op=ALU.add,
                )
                p0 = mid.tile([P, s], BF16, tag="p0", name="p0")
                nc.vector.tensor_tensor(
                    p0[:], tg[:, 0:s], logt[:][:, 0:s], op=ALU.mult
                )

                # PE: partition-reduce into rotating PSUM banks
                for which, lo, hi in mm_slices(s):
                    src = ff if which == "ff" else p0
                    b = mm_i % N_BANKS
                    nc.tensor.matmul(
                        loss_ps[b][:][:, 0 : hi - lo],
                        ones[:],
                        src[:][:, lo:hi],
                        start=(first_use[b] == mm_i),
                        stop=(last_use[b] == mm_i),
                    )
                    mm_i += 1

            # final PSUM -> SBUF copies on two engines in parallel, one DMA
            out_sb = accp.tile([1, (N_BANKS + 1) * MM_COLS], F32)
            nc.scalar.copy(out_sb[:, 0:MM_COLS], loss_ps[0][:])
            nc.vector.tensor_copy(
                out_sb[:, MM_COLS : 2 * MM_COLS], loss_ps[1][:]
            )
            nc.scalar.copy(
                out_sb[:, 2 * MM_COLS : 3 * MM_COLS], cnt_ps[:]
            )
            nc.sync.dma_start(out_d.ap(), out_sb[:])
    nc.compile()
    return nc


def pack_inputs(outputs, targets, order, widths, chunks):
    """fp16 staging + sorted variable-width v-plane packing per core."""
    o16 = np.ascontiguousarray(outputs).astype(np.float16)
    t16 = np.ascontiguousarray(targets).astype(np.float16)
    in_maps = []
    for c in range(N_CORES):
        m = {}
        for ci, ch in enumerate(chunks):
            s = sum(widths[j] for j in ch)
            ob = np.empty((P, 4, s), dtype=np.float16)
            tb = np.empty((P, 5, s), dtype=np.float16)
            off = 0
            for j in ch:
                w = widths[j]
                g = j * N_CORES + c
                rows = order[g * P : (g + 1) * P]
                ob[:, :, off : off + w] = o16[rows, :w, :].transpose(0, 2, 1)
                tb[:, :, off : off + w] = t16[rows, :w, :].transpose(0, 2, 1)
                off += w
            m[f"o_{ci}"] = ob.reshape(P, 4 * s)
            # planes order [t0 | t1..t4] already v-major via transpose
            m[f"t_{ci}"] = tb.reshape(P, 5 * s)
        in_maps.append(m)
    return in_maps


_NC_CACHE = {}


def _get_nc(widths, chunks):
    key = (tuple(widths), tuple(tuple(c) for c in chunks))
    if key not in _NC_CACHE:
        _NC_CACHE[key] = build_nc(widths, chunks)
    return _NC_CACHE[key]


def run_spmd(outputs, targets, trace=False, **kwargs):
    per_t = np.asarray(targets, dtype=np.float32).sum(axis=2)
    nz = per_t > 0
    lengths = np.where(nz.any(axis=1), T - nz[:, ::-1].argmax(axis=1), 0)
    order, widths, chunks = plan_schedule(lengths)
    in_maps = pack_inputs(outputs, targets, order, widths, chunks)
    nc = _get_nc(widths, chunks)
    res = run_bass_kernel_spmd(
        nc, in_maps, core_ids=list(range(N_CORES)), trace=trace, **kwargs
    )
    loss = sum(
        r["out_acc"][0, : N_BANKS * MM_COLS].astype(np.float64).sum()
        for r in res.results
    )
    cnt = sum(
        r["out_acc"][0, N_BANKS * MM_COLS :].astype(np.float64).sum()
        for r in res.results
    )
    return loss, cnt, res


def kernel(outputs, targets):
    loss, cnt, _ = run_spmd(outputs, targets)
    if cnt > 0:
        return np.float32(-loss / max(cnt, 1.0))
    return np.float32(0.0)
